# revision 24
# baseline (speedup 1.0000x reference)
"""Multi-head attention (b=2, s=2048, h=2048, 16 heads x 128) on 8 trn2 cores.

Sharding: core c handles batch c//4 and local head group c%4 (4 heads, 512
hidden cols). Per-core Bass kernel:
  A) three projection passes (q, k, v), each streaming x^T per 512-wide
     s-column with that projection's weights resident. q/k psum tiles are
     rope'd on eviction (partition-swap via SBUF->SBUF DMA + cos/sin DVE
     muls) straight into resident roped q^T/k^T tiles [d=128, s] -- no DRAM
     round trip. v is spilled to DRAM in natural [s, d_loc] layout.
  B) per head: scores^T = k^T-chunk x q^T into [j, i] layout so the alibi
     bias is per-partition and folds into the ACT exp; attn@v as
     lhsT=v-chunk, rhs=w' at N=512 producing ao^T [d, i] directly (no
     transpose); softmax denominator via ones-column matmul over w', then a
     k=1 broadcast matmul of 1/den, and one DVE mul to normalize into ao^T.
  C) row-parallel output projection -> per-core partial y [s, 2048].
Host: partial sums over each group of 4 cores -> full [2, s, 2048] output.

Matmuls run in float32r (full-rate PE, ~TF32 precision; rel err ~5e-4
end-to-end vs the 2e-2 gate); set KBENCH_F32R=0 for the 4x-slower exact
fp32 path. 1/sqrt(128) is folded into Wq host-side; rotate_half's sign is
folded into sinT host-side.
"""

import os

import numpy as np

import concourse.bass as bass
import concourse.mybir as mybir
import concourse.tile as tile
from concourse.bass_utils import run_bass_kernel_spmd

DT = mybir.dt.float32
FP = np.float32
S = 2048
HID = 2048
D = 128
NH = 4           # local heads per core
KC = HID // 128  # 16 k-chunks
JC = S // 128    # 16 j-chunks
IBLK = 4         # i-blocks of 512

TRACE = bool(int(os.environ.get("KBENCH_TRACE", "0")))
F32R = bool(int(os.environ.get("KBENCH_F32R", "1")))
DTM = mybir.dt.float32r if F32R else mybir.dt.float32  # matmul-input dtype
LAST_EXEC_NS = None
LAST_RESULTS = None

_NC = None
MAXW = 1  # this walrus build rejects >1 sem wait per instruction


def _split_excess_waits(nc, maxw=MAXW):
    """Hoist excess sem waits onto same-engine nofuse nops spliced in directly
    before the offending instruction. Pure condition hoisting: the engine
    blocks on each nop's waits before reaching the instruction, so semantics
    are identical."""
    for bb_name, bbw in list(nc.bb_map.items()):
        bb = bbw.bb if hasattr(bbw, "bb") else bbw
        insts = list(bb.instructions)
        changed = False
        out = []
        for inst in insts:
            si = inst.sync_info
            waits = list(si.on_wait) if si is not None and si.on_wait else []
            if len(waits) > maxw and inst.engine in nc.engines:
                si.on_wait = waits[:maxw]
                rest = waits[maxw:]
                for i in range(0, len(rest), maxw):
                    nop = nc.engines[inst.engine].nop(nofuse=True, hint="waitsplit")
                    cb = nc.cur_bb.bb
                    lst = list(cb.instructions)
                    assert lst[-1].name == nop.ins.name
                    cb.instructions = lst[:-1]
                    nop.ins.sync_info = mybir.SyncInfo(
                        on_wait=rest[i : i + maxw], on_update=[]
                    )
                    out.append(nop.ins)
                changed = True
            out.append(inst)
        if changed:
            bb.instructions = out


def build():
    nc = bass.Bass()
    xT = nc.declare_dram_parameter("xT", [HID, S], DTM, isOutput=False)
    wqT = nc.declare_dram_parameter("wqT", [HID, NH * D], DTM, isOutput=False)
    wkT = nc.declare_dram_parameter("wkT", [HID, NH * D], DTM, isOutput=False)
    wvT = nc.declare_dram_parameter("wvT", [HID, NH * D], DTM, isOutput=False)
    woT = nc.declare_dram_parameter("woT", [NH * D, HID], DTM, isOutput=False)
    cosT = nc.declare_dram_parameter("cosT", [D, S], DT, isOutput=False)
    sinT = nc.declare_dram_parameter("sinT", [D, S], DT, isOutput=False)
    alibi = nc.declare_dram_parameter("alibi", [128, NH * JC], DT, isOutput=False)
    ones = nc.declare_dram_parameter("ones", [128, 128], DTM, isOutput=False)
    y = nc.declare_dram_parameter("y", [S, HID], DT, isOutput=True)

    v_d = nc.dram_tensor("v_d", [S, NH * D], DTM)

    EXP = mybir.ActivationFunctionType.Exp

    def mm(out, lhsT, rhs, start=True, stop=True):
        nc.tensor.matmul(out, lhsT, rhs, start=start, stop=stop)

    with tile.TileContext(nc) as tc, nc.allow_low_precision(
        reason="f32r is 4-byte storage; matmul accum stays fp32 in PSUM"
    ):
        with (
            tc.tile_pool(name="psS", bufs=3, space="PSUM") as psS,
            tc.tile_pool(name="psN", bufs=2, space="PSUM") as psN,
            tc.tile_pool(name="psD", bufs=1, space="PSUM") as psD,
            tc.tile_pool(name="psB", bufs=2, space="PSUM") as psB,
            tc.tile_pool(name="persist", bufs=1) as persist,
            tc.tile_pool(name="xp", bufs=KC + 2) as xpool,
        ):
            ones_t = persist.tile([128, 128], DTM, tag="ones")
            nc.sync.dma_start(out=ones_t[:], in_=ones[:])
            al_t = persist.tile([128, NH * JC], DT, tag="al")
            nc.sync.dma_start(out=al_t[:], in_=alibi[:])
            qr_t = [
                persist.tile([D, S], DTM, tag=f"qr{h}", name=f"qr{h}")
                for h in range(NH)
            ]
            kr_t = [
                persist.tile([D, S], DTM, tag=f"kr{h}", name=f"kr{h}")
                for h in range(NH)
            ]

            # ---------------- Phase A: projections + fused rope ----------
            def load_w(pool, src, tg):
                ts_ = []
                for kc in range(KC):
                    t = pool.tile([128, NH * D], DTM, tag=tg, name=tg)
                    nc.sync.dma_start(out=t[:], in_=src[kc * 128 : (kc + 1) * 128, :])
                    ts_.append(t)
                return ts_

            def load_x_col(sc):
                xc = []
                for kc in range(KC):
                    t = xpool.tile([128, 512], DTM, tag="xc", name="xc")
                    nc.sync.dma_start(
                        out=t[:],
                        in_=xT[kc * 128 : (kc + 1) * 128, sc * 512 : (sc + 1) * 512],
                    )
                    xc.append(t)
                return xc

            with (
                tc.tile_pool(name="trig", bufs=1) as trig,
                tc.tile_pool(name="rp", bufs=2) as rp,
            ):
                cos_t = trig.tile([D, S], DT, tag="cos")
                nc.sync.dma_start(out=cos_t[:], in_=cosT[:])
                sin_t = trig.tile([D, S], DT, tag="sin")
                nc.sync.dma_start(out=sin_t[:], in_=sinT[:])

                def rope_evict(ps, dst, sc):
                    """dst[:, sc*512:+512] = rope(ps) where ps = raw projT psum."""
                    c0, c1 = sc * 512, (sc + 1) * 512
                    qtmp = rp.tile([128, 512], DT, tag="qtmp", name="qtmp")
                    nc.vector.tensor_copy(qtmp[:], ps[:])
                    qsw = rp.tile([128, 512], DT, tag="qsw", name="qsw")
                    nc.sync.dma_start(out=qsw[0:64, :], in_=qtmp[64:128, :])
                    nc.sync.dma_start(out=qsw[64:128, :], in_=qtmp[0:64, :])
                    rt = rp.tile([128, 512], DT, tag="rt", name="rt")
                    nc.vector.tensor_mul(rt[:], qsw[:], sin_t[:, c0:c1])
                    dsl = dst[:, c0:c1]
                    nc.vector.tensor_mul(dsl, qtmp[:], cos_t[:, c0:c1])
                    nc.vector.tensor_add(dsl, dsl, rt[:])

                for wparam, dst_tiles, tg in ((wqT, qr_t, "wq"), (wkT, kr_t, "wk")):
                    with tc.tile_pool(name=f"w{tg}", bufs=KC) as wp:
                        w_t = load_w(wp, wparam, tg)
                        for sc in range(4):
                            xc = load_x_col(sc)
                            for h in range(NH):
                                ps = psS.tile([128, 512], DT, tag="psA", name="psA")
                                for kc in range(KC):
                                    mm(
                                        ps[:],
                                        w_t[kc][:, h * D : (h + 1) * D],
                                        xc[kc][:],
                                        start=(kc == 0),
                                        stop=(kc == KC - 1),
                                    )
                                rope_evict(ps, dst_tiles[h], sc)

            with (
                tc.tile_pool(name="wwv", bufs=KC) as wvp,
                tc.tile_pool(name="stA", bufs=4) as stA,
            ):
                wv_t = load_w(wvp, wvT, "wv")
                for sc in range(4):
                    xc = load_x_col(sc)
                    for ss in range(4):
                        ps = psS.tile([128, 512], DT, tag="psA", name="psA")
                        for kc in range(KC):
                            mm(
                                ps[:],
                                xc[kc][:, ss * 128 : (ss + 1) * 128],
                                wv_t[kc][:],
                                start=(kc == 0),
                                stop=(kc == KC - 1),
                            )
                        st = stA.tile([128, 512], DTM, tag="stA", name="stA")
                        nc.vector.tensor_copy(st[:], ps[:])
                        nc.sync.dma_start(
                            out=v_d[sc * 512 + ss * 128 : sc * 512 + (ss + 1) * 128, :],
                            in_=st[:],
                        )

            # ---------------- Phase B: attention per head ----------------
            ao_t = []
            with (
                tc.tile_pool(name="vv", bufs=JC + 2) as vpool,
                tc.tile_pool(name="wexp", bufs=JC + 1) as wpool2,
                tc.tile_pool(name="sm", bufs=4) as smallpool,
            ):
                for h in range(NH):
                    ao = persist.tile([128, S], DTM, tag=f"ao{h}", name=f"ao{h}")
                    ao_t.append(ao)
                    vts = []
                    for jc in range(JC):
                        vt = vpool.tile([128, 128], DTM, tag="vt", name="vt")
                        nc.sync.dma_start(
                            out=vt[:],
                            in_=v_d[jc * 128 : (jc + 1) * 128, h * D : (h + 1) * D],
                        )
                        vts.append(vt)
                    for ib in range(IBLK):
                        i0, i1 = ib * 512, (ib + 1) * 512
                        wts_l = []
                        for jc in range(JC):
                            ps = psS.tile([128, 512], DT, tag="psA", name="psA")
                            mm(
                                ps[:],
                                kr_t[h][:, jc * 128 : (jc + 1) * 128],
                                qr_t[h][:, i0:i1],
                            )
                            w = wpool2.tile([128, 512], DTM, tag="w", name="w")
                            nc.scalar.activation(
                                w[:],
                                ps[:],
                                EXP,
                                bias=al_t[:, h * JC + jc : h * JC + jc + 1],
                                scale=1.0,
                            )
                            wts_l.append(w)
                        num = psN.tile([128, 512], DT, tag="num", name="num")
                        den = psD.tile([1, 512], DT, tag="den", name="den")
                        for jc in range(JC):
                            mm(
                                num[:],
                                vts[jc][:],
                                wts_l[jc][:],
                                start=(jc == 0),
                                stop=(jc == JC - 1),
                            )
                        for jc in range(JC):
                            mm(
                                den[:],
                                ones_t[:, 0:1],
                                wts_l[jc][:],
                                start=(jc == 0),
                                stop=(jc == JC - 1),
                            )
                        rec = smallpool.tile([1, 512], DTM, tag="rec", name="rec")
                        nc.vector.reciprocal(rec[:], den[:])
                        rb_ps = psB.tile([128, 512], DT, tag="rb", name="rb")
                        mm(rb_ps[:], ones_t[0:1, :], rec[:])
                        rb = smallpool.tile([128, 512], DT, tag="rbs", name="rbs")
                        nc.vector.tensor_copy(rb[:], rb_ps[:])
                        nc.vector.tensor_mul(ao[:, i0:i1], num[:], rb[:])

            # ---------------- Phase C: output projection ----------------
            with (
                tc.tile_pool(name="wo", bufs=1) as wopool,
                tc.tile_pool(name="stC", bufs=4) as stC,
            ):
                wo_t = []
                for cc in range(NH):
                    t = wopool.tile([128, HID], DTM, tag=f"wo{cc}", name=f"wo{cc}")
                    nc.sync.dma_start(out=t[:], in_=woT[cc * 128 : (cc + 1) * 128, :])
                    wo_t.append(t)
                for scn in range(S // 128):
                    for ocn in range(4):
                        ps = psS.tile([128, 512], DT, tag="psA", name="psA")
                        for cc in range(NH):
                            mm(
                                ps[:],
                                ao_t[cc][:, scn * 128 : (scn + 1) * 128],
                                wo_t[cc][:, ocn * 512 : (ocn + 1) * 512],
                                start=(cc == 0),
                                stop=(cc == NH - 1),
                            )
                        st = stC.tile([128, 512], DT, tag="stC", name="stC")
                        nc.vector.tensor_copy(st[:], ps[:])
                        nc.sync.dma_start(
                            out=y[
                                scn * 128 : (scn + 1) * 128, ocn * 512 : (ocn + 1) * 512
                            ],
                            in_=st[:],
                        )
    _split_excess_waits(nc)
    return nc


def _get_nc():
    global _NC
    if _NC is None:
        _NC = build()
    return _NC


def _numpy_fallback(x, attention_mask, alibi, freqs, Wq, Wk, Wv, Wo):
    b, s, hidden = x.shape
    H, d = 16, 128

    def proj(W):
        yv = x @ W.T
        return yv.reshape(b, s, H, d).transpose(0, 2, 1, 3)

    q, k, v = proj(Wq), proj(Wk), proj(Wv)
    cos, sin = np.cos(freqs), np.sin(freqs)

    def rot(t):
        t1, t2 = t[..., :64], t[..., 64:]
        return np.concatenate((-t2, t1), axis=-1)

    q = q * cos + rot(q) * sin
    k = k * cos + rot(k) * sin
    scores = np.einsum("bhqd,bhkd->bhqk", q, k) / np.sqrt(d)
    scores = scores + attention_mask + alibi
    m = scores.max(axis=-1, keepdims=True)
    e = np.exp(scores - m)
    attn = e / e.sum(axis=-1, keepdims=True)
    out = np.einsum("bhqk,bhkd->bhqd", attn, v)
    out = out.transpose(0, 2, 1, 3).reshape(b, s, hidden)
    return (out @ Wo.T).astype(np.float32)


def kernel(x, attention_mask, alibi, freqs, Wq, Wk, Wv, Wo):
    global LAST_EXEC_NS, LAST_RESULTS
    x = np.asarray(x, dtype=FP)
    attention_mask = np.asarray(attention_mask, dtype=FP)
    alibi = np.asarray(alibi, dtype=FP)
    freqs = np.asarray(freqs, dtype=FP)
    Wq, Wk, Wv, Wo = (np.asarray(w, dtype=FP) for w in (Wq, Wk, Wv, Wo))

    if np.any(attention_mask):
        return _numpy_fallback(x, attention_mask, alibi, freqs, Wq, Wk, Wv, Wo)

    nc = _get_nc()

    f = freqs.reshape(S, D)
    cosT = np.ascontiguousarray(np.cos(f).T)
    sinT = np.ascontiguousarray(np.sin(f).T)
    sinT[0:64, :] *= -1.0  # rotate_half sign folded into sin
    ones_np = np.ones((128, 128), dtype=FP)
    xTs = [np.ascontiguousarray(x[b].T) for b in range(2)]
    scale = FP(1.0 / np.sqrt(D))

    in_maps = []
    for c in range(8):
        b, g = divmod(c, 4)
        r0, r1 = g * 512, (g + 1) * 512
        wqT = np.ascontiguousarray(Wq[r0:r1, :].T) * scale
        wkT = np.ascontiguousarray(Wk[r0:r1, :].T)
        wvT = np.ascontiguousarray(Wv[r0:r1, :].T)
        woT = np.ascontiguousarray(Wo[:, r0:r1].T)
        al_loc = alibi[0, g * NH : (g + 1) * NH, 0, :]  # [4, 2048]
        al_dev = np.ascontiguousarray(
            al_loc.reshape(NH, JC, 128).transpose(2, 0, 1).reshape(128, NH * JC)
        )
        in_maps.append(
            {
                "xT": xTs[b],
                "wqT": wqT,
                "wkT": wkT,
                "wvT": wvT,
                "woT": woT,
                "cosT": cosT,
                "sinT": sinT,
                "alibi": al_dev,
                "ones": ones_np,
            }
        )

    res = run_bass_kernel_spmd(nc, in_maps, list(range(8)), trace=TRACE)
    LAST_EXEC_NS = res.exec_time_ns
    LAST_RESULTS = res
    ys = [res.results[c]["y"] for c in range(8)]
    out = np.stack(
        [ys[0] + ys[1] + ys[2] + ys[3], ys[4] + ys[5] + ys[6] + ys[7]], axis=0
    )
    return out.astype(np.float32)
